# revision 4
# baseline (speedup 1.0000x reference)
"""KGram MLP seq model (k-gram embedding lookup + 2-layer MLP + vocab projection)
on 8 Trainium2 NeuronCores.

Data-parallel over the S*B = 4096 token positions (512 rows/core; cores 0-3
take batch 0, cores 4-7 batch 1, each owning 512 contiguous sequence
positions).  All weights replicated per core.  fp16 pipeline with an fp8
(e4m3) DoubleRow pair on the last two contraction blocks of the vocab
projection.  Per core:

  1. indirect-DMA gather of the (T + K - 1) embedding rows (token-major)
  2. PE transposes (matmul w/ identity) -> feature-major G^T tiles; PE kept
     warm with filler matmuls so the HAM clock gate stays at 2.4 GHz
  3. h1^T = silu(W1^T x^T + b1)  (three K-blocks = shifted column windows)
  4. h2^T = silu(W2^T h1^T + b2), then h2s = h2 * 2^13 (exact in fp16);
     blocks 6,7 of h2s also cast to e4m3 for the fp8 pair
  5. logits^T: per 128-vocab block one PSUM group accumulates 6 fp16 matmuls
     (d-blocks 0-5, weights pre-scaled x2^11 on host) plus one fp8 DoubleRow
     matmul covering d-blocks 6,7 (weights x2^11 in e4m3, h2 x2^13); the
     scalar-engine drain descales by 2^-24 and adds bout:
     out = psum * 2^-24 + bo -> fp16, streamed to HBM on the sync queue

Host converts fp16 logits to f32 and reassembles (S, B, V).
"""

import math

import numpy as np
import ml_dtypes

import concourse.bass as bass
import concourse.mybir as mybir
import concourse.tile as tile
from concourse import bacc
from concourse.bass_utils import run_bass_kernel_spmd

P = 128
NCORES = 8

VOCAB = 50257
EMBED = 1024
SEQ = 2048
BATCH = 2
KGRAM = 3
VPAD = 50304   # 393 * 128
MGROUP = 2048  # vocab columns per Wout streaming group

USE_FP8 = True  # d-blocks 6,7 of the vocab projection as one fp8 pair
SH = 8192.0     # h2 scale (2^13): fp8 range use; exact (power of 2) in fp16
SW = 2048.0     # Wout scale (2^11)

_nc_cache: dict = {}


def _build(V, D, KC, T, VP, MG, use_fp8):
    DK = D // P
    TW = T + KC - 1
    NG = math.ceil(TW / P)
    NM = VP // P
    NDF = DK - 2 if use_fp8 else DK  # fp16 d-blocks in the vocab projection
    descale = 1.0 / (SH * SW) if use_fp8 else 1.0
    f32 = mybir.dt.float32
    f16 = mybir.dt.float16
    e4 = mybir.dt.float8e4
    i32 = mybir.dt.int32
    AF = mybir.ActivationFunctionType

    nc = bacc.Bacc()

    E_d = nc.declare_dram_parameter("E", [V, D], f16, isOutput=False)
    W1_d = nc.declare_dram_parameter("W1", [KC * D, D], f16, isOutput=False)
    W2_d = nc.declare_dram_parameter("W2", [D, D], f16, isOutput=False)
    Wo_d = nc.declare_dram_parameter("Wo", [NDF * P, VP], f16, isOutput=False)
    if use_fp8:
        Wo8_d = nc.declare_dram_parameter("Wo8", [P, 2, VP], e4, isOutput=False)
    b1_d = nc.declare_dram_parameter("b1", [P, DK], f32, isOutput=False)
    b2_d = nc.declare_dram_parameter("b2", [P, DK], f32, isOutput=False)
    bo_d = nc.declare_dram_parameter("bo", [P, NM], f32, isOutput=False)
    tok_d = nc.declare_dram_parameter("toks", [P, NG], i32, isOutput=False)
    id_d = nc.declare_dram_parameter("ident", [P, P], f16, isOutput=False)
    out_d = nc.declare_dram_parameter("out", [VP, T], f16, isOutput=True)

    grows = []
    for g in range(NG):
        rows = min(P, TW - g * P)
        grows.append(((rows + 15) // 16) * 16)  # DMA row granularity

    with tile.TileContext(nc) as tc:
        with (
            tc.tile_pool(name="const", bufs=1) as cpool,
            tc.tile_pool(name="gath", bufs=1) as gpool,
            tc.tile_pool(name="gt", bufs=1) as gtpool,
            tc.tile_pool(name="w", bufs=1) as wpool,
            tc.tile_pool(name="h", bufs=1) as hpool,
            tc.tile_pool(name="wo", bufs=2) as wopool,
            tc.tile_pool(name="ot", bufs=4) as opool,
            tc.tile_pool(name="psM", bufs=6, space="PSUM") as psM,
            tc.tile_pool(name="psT", bufs=2, space="PSUM") as psT,
        ):
            # token indices first so the gathers can start immediately
            tok_s = cpool.tile([P, NG], i32, tag="tok")
            nc.sync.dma_start(tok_s[:], tok_d[:])
            ident = cpool.tile([P, P], f16, tag="ident")
            nc.sync.dma_start(ident[:], id_d[:])
            b1_s = cpool.tile([P, DK], f32, tag="b1")
            nc.sync.dma_start(b1_s[:], b1_d[:])
            b2_s = cpool.tile([P, DK], f32, tag="b2")
            nc.sync.dma_start(b2_s[:], b2_d[:])
            bo_s = cpool.tile([P, NM], f32, tag="bo")
            nc.sync.dma_start(bo_s[:], bo_d[:])

            # --- embedding gather (token-major), all gathers queued up front ---
            gtiles = []
            for g in range(NG):
                gtile = gpool.tile([P, D], f16, tag=f"g{g}", name=f"g{g}")
                nc.gpsimd.indirect_dma_start(
                    out=gtile[: grows[g], :],
                    out_offset=None,
                    in_=E_d[:],
                    in_offset=bass.IndirectOffsetOnAxis(
                        ap=tok_s[: grows[g], g : g + 1], axis=0
                    ),
                )
                gtiles.append(gtile)

            # W1/W2 loads early on the sync queue (needed at ~30us)
            w1s = []
            for kc in range(KC * DK):
                t = wpool.tile([P, D], f16, tag=f"w1_{kc}", name=f"w1_{kc}")
                nc.sync.dma_start(t[:], W1_d[kc * P : (kc + 1) * P, :])
                w1s.append(t)
            w2s = []
            for kc in range(DK):
                t = wpool.tile([P, D], f16, tag=f"w2_{kc}", name=f"w2_{kc}")
                nc.sync.dma_start(t[:], W2_d[kc * P : (kc + 1) * P, :])
                w2s.append(t)

            # PE warmup: fill the HAM activity window while gathers fly
            warm = cpool.tile([P, P], f16, tag="warm")
            nc.vector.memset(warm[:], 0.5)
            warm_r = cpool.tile([P, T], f16, tag="warm_r")
            nc.vector.memset(warm_r[:], 0.5)

            def warmup(n):
                for _ in range(n):
                    ps = psM.tile([P, T], f32, tag="mm", name="warm_ps")
                    nc.tensor.matmul(ps[:], lhsT=warm[:], rhs=warm_r[:],
                                     start=True, stop=True)

            warmup(31)

            # --- transpose to feature-major G^T on the PE (keeps HAM warm) ---
            gts = [
                gtpool.tile([P, NG * P], f16, tag=f"gt{f}", name=f"gt{f}")
                for f in range(DK)
            ]
            for g in range(NG):
                r = grows[g]
                for f in range(DK):
                    pst = psT.tile([P, P], f16, tag="tr", name="pst")
                    nc.tensor.transpose(
                        pst[:, :r],
                        gtiles[g][:r, f * P : (f + 1) * P],
                        ident[:r, :r],
                    )
                    nc.scalar.activation(
                        gts[f][:, g * P : g * P + r], pst[:, :r], AF.Identity
                    )
                if g < NG - 1:
                    warmup((6, 7, 8, 10)[g])

            # --- MLP layer 1: h1^T = silu(W1^T x^T + b1) ---
            h1 = [
                hpool.tile([P, T], f16, tag=f"h1_{m}", name=f"h1_{m}")
                for m in range(DK)
            ]
            for m in range(DK):
                ps = psM.tile([P, T], f32, tag="mm", name="l1ps")
                n = 0
                for i in range(KC):
                    for k8 in range(DK):
                        kc = i * DK + k8
                        nc.tensor.matmul(
                            ps[:],
                            lhsT=w1s[kc][:, m * P : (m + 1) * P],
                            rhs=gts[k8][:, i : i + T],
                            start=(n == 0),
                            stop=(n == KC * DK - 1),
                        )
                        n += 1
                nc.scalar.activation(h1[m][:], ps[:], AF.Silu, bias=b1_s[:, m : m + 1])

            # --- MLP layer 2: h2^T = silu(W2^T h1^T + b2), then x2^13 ---
            h2 = [
                hpool.tile([P, T], f16, tag=f"h2_{m}", name=f"h2_{m}")
                for m in range(DK)
            ]
            h2s = h2
            for m in range(DK):
                ps = psM.tile([P, T], f32, tag="mm", name="l2ps")
                for k8 in range(DK):
                    nc.tensor.matmul(
                        ps[:],
                        lhsT=w2s[k8][:, m * P : (m + 1) * P],
                        rhs=h1[k8][:],
                        start=(k8 == 0),
                        stop=(k8 == DK - 1),
                    )
                nc.scalar.activation(h2[m][:], ps[:], AF.Silu, bias=b2_s[:, m : m + 1])

            if use_fp8:
                # scaled copies: exact in fp16 (power of 2); fp8 needs range
                h2s = [
                    hpool.tile([P, T], f16, tag=f"h2s_{m}", name=f"h2s_{m}")
                    for m in range(DK)
                ]
                for m in range(DK):
                    nc.vector.tensor_scalar_mul(h2s[m][:], h2[m][:], SH)
                h2f8 = hpool.tile([P, 2, T], e4, tag="h2f8")
                for i in range(2):
                    nc.vector.tensor_scalar_mul(h2f8[:, i, :], h2s[NDF + i][:], 1.0)

            # --- vocab projection ---
            c0 = 0
            while c0 < VP:
                cols = min(MG, VP - c0)
                wos = []
                for k8 in range(NDF):
                    t = wopool.tile([P, MG], f16, tag=f"wo{k8}", name=f"wo{k8}")
                    nc.sync.dma_start(
                        t[:, :cols], Wo_d[k8 * P : (k8 + 1) * P, c0 : c0 + cols]
                    )
                    wos.append(t)
                if use_fp8:
                    wo8 = wopool.tile([P, 2, MG], e4, tag="wo8", name="wo8")
                    nc.sync.dma_start(
                        wo8[:, :, :cols], Wo8_d[:, :, c0 : c0 + cols]
                    )
                for m in range(cols // P):
                    ps = psM.tile([P, T], f32, tag="mm", name="wops")
                    for k8 in range(NDF):
                        nc.tensor.matmul(
                            ps[:],
                            lhsT=wos[k8][:, m * P : (m + 1) * P],
                            rhs=h2s[k8][:],
                            start=(k8 == 0),
                            stop=(not use_fp8 and k8 == NDF - 1),
                        )
                    if use_fp8:
                        nc.tensor.matmul(
                            ps[:],
                            lhsT=wo8[:, :, m * P : (m + 1) * P],
                            rhs=h2f8[:, :, :],
                            start=False,
                            stop=True,
                            perf_mode=mybir.MatmulPerfMode.DoubleRow,
                        )
                    ot = opool.tile([P, T], f16, tag="ot")
                    mi = (c0 + m * P) // P
                    nc.scalar.activation(
                        ot[:], ps[:], AF.Identity,
                        bias=bo_s[:, mi : mi + 1], scale=descale,
                    )
                    nc.sync.dma_start(
                        out_d[c0 + m * P : c0 + (m + 1) * P, :], ot[:]
                    )
                c0 += cols

    nc.finalize()
    return nc


def _get_nc(V, D, KC, T, VP, MG, use_fp8):
    key = (V, D, KC, T, VP, MG, use_fp8)
    if key not in _nc_cache:
        _nc_cache[key] = _build(V, D, KC, T, VP, MG, use_fp8)
    return _nc_cache[key]


def _run(tokens, E, W1, b1, W2, b2, Wout, bout, V, D, KC, VP, MG, trace=False,
         tmpdir=None, use_fp8=USE_FP8):
    """tokens: (S, B) int32.  Returns (S, B, V) f32 logits (and results obj)."""
    f16 = np.float16
    S, B = tokens.shape
    cpb = NCORES // B  # cores per batch column
    T = S // cpb
    DK = D // P
    TW = T + KC - 1
    NG = math.ceil(TW / P)
    TWPAD = NG * P
    NM = VP // P
    NDF = DK - 2 if use_fp8 else DK
    sw = SW if use_fp8 else 1.0

    E_h = E.astype(f16)
    W1_h = W1.astype(f16)
    W2_h = W2.astype(f16)
    Wo_h = np.zeros((NDF * P, VP), dtype=f16)
    Wo_h[:, :V] = (Wout[: NDF * P, :] * sw).astype(f16)
    b1t = np.ascontiguousarray(b1.reshape(DK, P).T.astype(np.float32))
    b2t = np.ascontiguousarray(b2.reshape(DK, P).T.astype(np.float32))
    bo_p = np.zeros(VP, dtype=np.float32)
    bo_p[:V] = bout
    bot = np.ascontiguousarray(bo_p.reshape(NM, P).T)
    ident = np.eye(P, dtype=f16)
    if use_fp8:
        Wo8_h = np.zeros((P, 2, VP), dtype=ml_dtypes.float8_e4m3)
        w8 = (Wout[NDF * P :, :].astype(np.float32) * SW).reshape(2, P, V)
        Wo8_h[:, :, :V] = w8.transpose(1, 0, 2).astype(ml_dtypes.float8_e4m3)

    nc = _get_nc(V, D, KC, T, VP, MG, use_fp8)

    in_maps = []
    for c in range(NCORES):
        b, chunk = divmod(c, cpb)
        s0 = chunk * T
        pad = np.zeros(TWPAD, dtype=np.int32)
        lo = max(0, s0 - (KC - 1))
        seg = tokens[lo : s0 + T, b]
        start = (KC - 1) - (s0 - lo)
        pad[start : start + seg.size] = seg
        tok2d = np.ascontiguousarray(pad.reshape(NG, P).T)
        m = {
            "E": E_h,
            "W1": W1_h,
            "W2": W2_h,
            "Wo": Wo_h,
            "b1": b1t,
            "b2": b2t,
            "bo": bot,
            "toks": tok2d,
            "ident": ident,
        }
        if use_fp8:
            m["Wo8"] = Wo8_h
        in_maps.append(m)

    kres = run_bass_kernel_spmd(
        nc, in_maps, list(range(NCORES)), trace=trace, tmpdir=tmpdir
    )
    res = kres.results

    out = np.empty((S, B, V), dtype=np.float32)
    for c in range(NCORES):
        b, chunk = divmod(c, cpb)
        s0 = chunk * T
        out[s0 : s0 + T, b, :] = res[c]["out"][:V, :].astype(np.float32).T
    return out, kres


def kernel(**inputs):
    tokens = np.asarray(inputs["tokens_seq"]).astype(np.int32)
    E = np.asarray(inputs["E"], dtype=np.float32)
    W1 = np.asarray(inputs["W1"], dtype=np.float32)
    b1 = np.asarray(inputs["b1"], dtype=np.float32)
    W2 = np.asarray(inputs["W2"], dtype=np.float32)
    b2 = np.asarray(inputs["b2"], dtype=np.float32)
    Wout = np.asarray(inputs["Wout"], dtype=np.float32)
    bout = np.asarray(inputs["bout"], dtype=np.float32)
    out, _ = _run(
        tokens, E, W1, b1, W2, b2, Wout, bout,
        V=VOCAB, D=EMBED, KC=KGRAM, VP=VPAD, MG=MGROUP,
    )
    return out
